# revision 21
# baseline (speedup 1.0000x reference)
"""Causal self-attention (nn_CausalSelfAttention) on 8 TRN2 NeuronCores.

Reference computation (B=2, T=2048, C=1024, H=16 heads, D=64):
    qkv = x @ W_attn.T + b_attn ; split q,k,v
    y   = softmax(causal(q k^T / sqrt(D))) v        (per head)
    out = y @ W_proj.T + b_proj

Sharding: batch (2-way) x head-group (4-way, 4 heads each) -> 8 cores.
Each core computes its batch's attention for its 4 heads plus the partial
c_proj contribution of those heads' channels; the host sums the 4 partials
per batch (fp32) and adds the (adjusted) bias once.

Device-side simplifications (exact up to fp error):
  - k bias dropped (cancels in softmax); v bias folded into the host-side
    output bias (sum(P)=1 per row).

v3 design (PE tile-position concurrency + geometry-uniform batching):
  - S matmuls are K=64: the two sub-heads of a pair run as row tiles
    (0,0)/(64,0) adjacently -> ~2x PE throughput; per head-pair (pp) the
    S pair writes a [128, 2*512] two-bank PSUM tile and ONE fused ACT exp
    covers both heads (halves the ~293ns/instr ACT overhead). The two
    pps are staggered so S of one pp hides under the other pp's exp.
  - PV is M=64 col tiles (0,0)/(0,64) accumulating head pairs into one
    PSUM bank; denominators are 4x M=32 col-tiled chains (ones
    stationary) in one bank. PV+den are emitted in CHUNKS of 4 rounds:
    a chunk is a geometry-uniform burst (all deps long done -> single
    readiness wave), so the expensive PE geometry transitions (exposed
    drain + LDWEIGHTS, ~300ns each) are paid per chunk, not per round.
  - Production/projection chains drip between rounds; strip-0 production
    rotates over the (then idle) PV+den banks to double-buffer drains.
  - Final projection rotates over the S pair banks (idle by then), with
    drains split across Vector+Scalar; output is bf16 to halve the
    output DMA.

PSUM (8 banks): S pair tiles 2x2 + PV pairs 2 + denom 1 + production 1.
Known pitfalls: an accumulation chain covers all bytes it touches on its
first (start=True) matmul; custom DVE/gpsimd ops read partition 0 of
their input AP regardless of its base.
"""
import math
from contextlib import ExitStack

import ml_dtypes
import numpy as np

import concourse.bacc as bacc
import concourse.bass as bass
import concourse.mybir as mybir
import concourse.tile as tile
from concourse.bass_utils import run_bass_kernel_spmd

F32 = mybir.dt.float32
BF16 = mybir.dt.bfloat16
MMDT = BF16                    # dtype for all TensorE-facing tensors

N_CORES = 8
B, T, C, H = 2, 2048, 1024, 16
D = 64
GROUPS = N_CORES // B          # head groups per batch = 4
HPC = H // GROUPS              # heads per core = 4
CS = HPC * D                   # channel slice per core = 256
KT = C // 128                  # contraction tiles over C = 8
NS = T // 512                  # 512-wide query strips = 4
TT = T // 128                  # 128-row key tiles = 16
CH = 4                         # PV/den chunk size in rounds


def build_nc():
    nc = bacc.Bacc("TRN2", target_bir_lowering=False, debug=False,
                   num_devices=N_CORES)

    xT = nc.dram_tensor("xT", [C, T], MMDT, kind="ExternalInput")
    wqkT = nc.dram_tensor("wqkT", [C, 2 * CS], MMDT, kind="ExternalInput")
    bq = nc.dram_tensor("bq", [128, 2], F32, kind="ExternalInput")
    wvT = nc.dram_tensor("wvT", [C, CS], MMDT, kind="ExternalInput")
    wpT = nc.dram_tensor("wpT", [CS, C], MMDT, kind="ExternalInput")
    outT = nc.dram_tensor("outT", [C, T], BF16, kind="ExternalOutput")

    xTr = xT.ap().rearrange("(kt p) t -> kt p t", p=128)
    wqkr = wqkT.ap().rearrange("(kt p) n -> kt p n", p=128)
    wvr = wvT.ap().rearrange("(kt p) n -> kt p n", p=128)
    wpr = wpT.ap().rearrange("(kt p) n -> kt p n", p=128)

    scale = 1.0 / math.sqrt(D)

    with tile.TileContext(nc) as tc, ExitStack() as ctx:
        pw = ctx.enter_context(tc.tile_pool(name="pw", bufs=1))
        px = ctx.enter_context(tc.tile_pool(name="px", bufs=1))
        pq = ctx.enter_context(tc.tile_pool(name="pq", bufs=1))
        pk = ctx.enter_context(tc.tile_pool(name="pk", bufs=1))
        pv = ctx.enter_context(tc.tile_pool(name="pv", bufs=1))
        py = ctx.enter_context(tc.tile_pool(name="py", bufs=1))
        ppt = ctx.enter_context(tc.tile_pool(name="ppt", bufs=14))
        pnorm = ctx.enter_context(tc.tile_pool(name="pnorm", bufs=4))
        pout = ctx.enter_context(tc.tile_pool(name="pout", bufs=4))
        psS = ctx.enter_context(tc.tile_pool(name="psS", bufs=1, space="PSUM"))
        psPV = ctx.enter_context(tc.tile_pool(name="psPV", bufs=1,
                                              space="PSUM"))
        psD = ctx.enter_context(tc.tile_pool(name="psD", bufs=1, space="PSUM"))
        psP = ctx.enter_context(tc.tile_pool(name="psP", bufs=1, space="PSUM"))

        # ---- input DMA: per-tile 2D transfers, x strip0 + wqk first ----
        qs = [nc.sync, nc.scalar, nc.gpsimd]
        xq = [[None] * KT for _ in range(2)]   # [s][k] quarters, s in 0,1
        xh = [None] * KT                       # [k] cols 1024:2048
        wqk_sb, wv_sb = [], []
        qi = 0
        for k in range(KT):
            t_ = px.tile([128, 512], MMDT, tag=f"xq{k}_0", name=f"xq{k}_0")
            qs[qi % 3].dma_start(t_[:], xTr[k][:, 0:512])
            qi += 1
            xq[0][k] = t_
            wt = pw.tile([128, 2 * CS], MMDT, tag=f"wqk{k}", name=f"wqk{k}")
            qs[qi % 3].dma_start(wt[:], wqkr[k])
            qi += 1
            wqk_sb.append(wt)
        bq_sb = pw.tile([128, 2], F32, tag="bq", name="bq_sb")
        nc.gpsimd.dma_start(bq_sb[:], bq.ap())
        warm = pnorm.tile([128, 1], F32, tag="warm", name="warm")
        nc.scalar.activation(warm[:], bq_sb[:, 0:1],
                             mybir.ActivationFunctionType.Exp, scale=0.0)
        for k in range(KT):
            vt = pw.tile([128, CS], MMDT, tag=f"wv{k}", name=f"wv{k}")
            qs[(k + 1) % 3].dma_start(vt[:], wvr[k])
            wv_sb.append(vt)
        for k in range(KT):
            t_ = px.tile([128, 512], MMDT, tag=f"xq{k}_1", name=f"xq{k}_1")
            qs[k % 3].dma_start(t_[:], xTr[k][:, 512:1024])
            xq[1][k] = t_
        wp_sb = []
        for k2 in range(2):
            pt_ = pw.tile([128, C], MMDT, tag=f"wp{k2}", name=f"wp{k2}")
            nc.gpsimd.dma_start(pt_[:], wpr[k2])
            wp_sb.append(pt_)
        for k in range(KT):
            t_ = px.tile([128, 1024], MMDT, tag=f"xh{k}", name=f"xh{k}")
            qs[k % 2].dma_start(t_[:], xTr[k][:, 1024:2048])
            xh[k] = t_

        ones_sb = pw.tile([128, 32], MMDT, tag="ones", name="ones_sb")
        nc.vector.memset(ones_sb[:], 1.0)

        def x_strip(k, s):
            if s < 2:
                return xq[s][k][:]
            return xh[k][:, (s - 2) * 512:(s - 1) * 512]

        # persistent SBUF tensors (pair layout: sub-head r at partitions
        # r*64..(r+1)*64)
        q_sb = [pq.tile([128, T], MMDT, tag=f"q{pp}", name=f"q{pp}")
                for pp in range(2)]
        k_sb = [pk.tile([128, T], MMDT, tag=f"k{pp}", name=f"k{pp}")
                for pp in range(2)]
        v_sb = [pv.tile([128, CS], MMDT, tag=f"v{n}", name=f"v{n}")
                for n in range(TT)]
        y_sb = [py.tile([128, T], MMDT, tag=f"y{pp}", name=f"y{pp}")
                for pp in range(2)]

        def unit_k(s, pp, pool=None, tag="pm"):
            ps = (pool or psP).tile([128, 512], F32, tag=tag, name="ps_k")
            steps = []
            for k in range(KT):
                steps.append(lambda k=k, ps=ps: nc.tensor.matmul(
                    ps[:],
                    wqk_sb[k][:, (2 + pp) * 128:(3 + pp) * 128],
                    x_strip(k, s),
                    start=(k == 0), stop=(k == KT - 1),
                ))
            steps.append(lambda ps=ps: nc.vector.tensor_copy(
                k_sb[pp][:, s * 512:(s + 1) * 512], ps[:]))
            return steps

        def unit_q(s, pp, pool=None, tag="pm"):
            ps = (pool or psP).tile([128, 512], F32, tag=tag, name="ps_q")
            steps = []
            for k in range(KT):
                steps.append(lambda k=k, ps=ps: nc.tensor.matmul(
                    ps[:],
                    wqk_sb[k][:, pp * 128:(pp + 1) * 128],
                    x_strip(k, s),
                    start=(k == 0), stop=(k == KT - 1),
                ))
            steps.append(lambda ps=ps: nc.vector.tensor_scalar(
                q_sb[pp][:, s * 512:(s + 1) * 512], ps[:],
                bq_sb[:, pp:pp + 1], None, op0=mybir.AluOpType.add))
            return steps

        def unit_v(s, j, pool=None, tag="pm"):
            n = 4 * s + j
            ps = (pool or psP).tile([128, 512], F32, tag=tag, name="ps_v")
            steps = []
            for k in range(KT):
                steps.append(lambda k=k, ps=ps: nc.tensor.matmul(
                    ps[:, 0:CS],
                    x_strip(k, s)[:, j * 128:(j + 1) * 128],
                    wv_sb[k][:],
                    start=(k == 0), stop=(k == KT - 1),
                ))
            steps.append(lambda ps=ps, n=n: nc.vector.tensor_copy(
                v_sb[n][:], ps[:, 0:CS]))
            return steps

        def unit_proj(s, ct, pool=None, tag="pm", split_drain=False):
            ps = (pool or psP).tile([128, 512], F32, tag=tag, name="ps_o")
            steps = []
            for k2 in range(2):
                steps.append(lambda k2=k2, ps=ps: nc.tensor.matmul(
                    ps[:],
                    wp_sb[k2][:, ct * 128:(ct + 1) * 128],
                    y_sb[k2][:, s * 512:(s + 1) * 512],
                    start=(k2 == 0), stop=(k2 == 1),
                ))

            def drain(ps=ps, s=s, ct=ct):
                ot = pout.tile([128, 512], BF16, tag="ot", name="ot")
                if split_drain:
                    nc.vector.tensor_copy(ot[:, 0:256], ps[:, 0:256])
                    nc.scalar.copy(ot[:, 256:512], ps[:, 256:512])
                    eng = [nc.sync, nc.scalar][ct % 2]
                else:
                    nc.vector.tensor_copy(ot[:], ps[:])
                    eng = nc.sync if ct % 2 == 0 else nc.gpsimd
                eng.dma_start(
                    outT.ap()[ct * 128:(ct + 1) * 128,
                              s * 512:(s + 1) * 512],
                    ot[:])
            steps.append(drain)
            return steps

        def kq_units(s, pools=None):
            us, pl, i = [], pools or [(None, "pm")], 0
            for pp in range(2):
                po_, tg = pl[i % len(pl)]; i += 1
                us.append(unit_k(s, pp, pool=po_, tag=tg))
            for pp in range(2):
                po_, tg = pl[i % len(pl)]; i += 1
                us.append(unit_q(s, pp, pool=po_, tag=tg))
            return us

        def v_units(s, pools=None):
            us, pl, i = [], pools or [(None, "pm")], 0
            for j in range(4):
                po_, tg = pl[i % len(pl)]; i += 1
                us.append(unit_v(s, j, pool=po_, tag=tg))
            return us

        def run_units(units, count):
            done = 0
            while units and done < count:
                for f in units.pop(0):
                    f()
                done += 1

        # strip-0 production rotates over psP + (idle) psPV/psD banks
        s0pools = [(psP, "pm"), (psPV, "pv0"), (psPV, "pv1"), (psD, "den")]
        for u in kq_units(0, pools=s0pools) + v_units(0, pools=s0pools):
            for f in u:
                f()

        for s in range(NS):
            # drip load-balanced by strip capacity: v(s) lands in this
            # strip's first rounds (first PV chunk needs it at round 4),
            # kq(s+1) by strip end, projection deferred to late strips.
            bg = []
            if s >= 1:
                bg += v_units(s)
            if s + 1 < NS:
                pools = s0pools if s == 0 else None
                bg += kq_units(s + 1, pools=pools)
            if s == 2:
                bg += [unit_proj(0, ct) for ct in range(8)]
            if s == 3:
                bg += [unit_proj(1, ct) for ct in range(8)]
                bg += [unit_proj(2, ct) for ct in range(8)]
            nbg = len(bg)
            nt = 4 * s + 4
            pv_ps = [psPV.tile([128, 512], F32, tag=f"pv{pp}",
                               name=f"pv{pp}") for pp in range(2)]
            den_ps = psD.tile([128, 512], F32, tag="den", name="den_ps")
            emitted = 0
            pend = []          # (n, off, pts) awaiting chunk emission

            def emit_chunk(keep=0):
                # geometry-uniform burst: all PV col-tile pairs, then all
                # den groups; every dep (exp) completed rounds ago. `keep`
                # holds back the freshest rounds whose exps may still be
                # in flight.
                batch = pend[:len(pend) - keep] if keep else pend[:]
                del pend[:len(batch)]
                for (n, off, pts) in batch:
                    for h4 in range(4):
                        nc.tensor.matmul(
                            den_ps[32 * h4:32 * (h4 + 1), off:512],
                            ones_sb[:],
                            pts[h4 // 2][:, h4 % 2, off:512],
                            start=(n == 0), stop=(n == nt - 1),
                            tile_position=(0, 32 * h4),
                        )
                for (n, off, pts) in batch:
                    for pp in range(2):
                        for r in range(2):
                            nc.tensor.matmul(
                                pv_ps[pp][r * 64:(r + 1) * 64, off:512],
                                v_sb[n][:, (2 * pp + r) * 64:
                                        (2 * pp + r + 1) * 64],
                                pts[pp][:, r, off:512],
                                start=(n == 0), stop=(n == nt - 1),
                            )

            for n in range(nt):
                if (n % CH == 0 and n > 0) or n == nt - 1:
                    emit_chunk(keep=2)
                target = (n * nbg) // nt if s == 0 else ((n + 1) * nbg) // nt
                run_units(bg, target - emitted)
                emitted = min(target, nbg)
                off = max(0, n - 4 * s) * 128
                pts = []
                for pp in range(2):
                    st = psS.tile([128, 2 * 512], F32, tag=f"s{pp}",
                                  name=f"st{pp}")
                    st3 = st.rearrange("p (r m) -> p r m", m=512)
                    for r in range(2):
                        nc.tensor.matmul(
                            st3[:, r, off:512],
                            k_sb[pp][r * 64:(r + 1) * 64,
                                     n * 128:(n + 1) * 128],
                            q_sb[pp][r * 64:(r + 1) * 64,
                                     s * 512 + off:(s + 1) * 512],
                            start=True, stop=True,
                        )
                    pt = ppt.tile([128, 2 * 512], MMDT, tag="pt", name="pt")
                    pt3 = pt.rearrange("p (r m) -> p r m", m=512)
                    nc.scalar.activation(
                        pt3[:, :, off:512], st3[:, :, off:512],
                        mybir.ActivationFunctionType.Exp, scale=scale)
                    if n >= 4 * s:
                        for r in range(2):
                            nc.gpsimd.affine_select(
                                out=pt3[:, r, off:off + 128],
                                in_=pt3[:, r, off:off + 128],
                                compare_op=mybir.AluOpType.is_ge,
                                fill=0.0, base=0,
                                pattern=[[1, 128]], channel_multiplier=-1)
                    pts.append(pt3)
                pend.append((n, off, pts))

            emit_chunk()
            run_units(bg, 10**9)

            # ---- normalize: y = y_unnorm / denom ----
            rrs, rbs = [], []
            for h4 in range(4):
                dtmp = pnorm.tile([1, 512], F32, tag="dtmp", name="dtmp")
                nc.vector.tensor_copy(dtmp[:], den_ps[32 * h4:32 * h4 + 1, :])
                rr = pnorm.tile([1, 512], F32, tag="rr", name="rr")
                nc.vector.reciprocal_approx_fast(rr[:], dtmp[:])
                rrs.append(rr)
            for pp in range(2):
                for r in range(2):
                    h4 = 2 * pp + r
                    rb = pnorm.tile([64, 512], F32, tag="rb", name="rb")
                    nc.gpsimd.partition_broadcast(rb[:], rrs[h4][:])
                    rbs.append(rb)
                for r in range(2):
                    h4 = 2 * pp + r
                    nc.vector.tensor_tensor(
                        y_sb[pp][r * 64:(r + 1) * 64, s * 512:(s + 1) * 512],
                        pv_ps[pp][r * 64:(r + 1) * 64, :], rbs[h4][:],
                        op=mybir.AluOpType.mult)

        # final projection: rotate over S pair banks (idle now) + psP
        tailpools = [(psS, "s0"), (psS, "s1"), (psP, "pm")]
        for ct in range(8):
            po_, tg = tailpools[ct % 3]
            for f in unit_proj(NS - 1, ct, pool=po_, tag=tg,
                               split_drain=True):
                f()

    nc.compile()
    return nc


def make_in_maps(x, W_attn, b_attn, W_proj):
    """Shard full inputs into the 8 per-core input dicts."""
    x = np.asarray(x, dtype=np.float32)
    W_attn = np.asarray(W_attn, dtype=np.float32)
    b_attn = np.asarray(b_attn, dtype=np.float32)
    W_proj = np.asarray(W_proj, dtype=np.float32)
    in_maps = []
    xTb = [np.ascontiguousarray(x[b_].T) for b_ in range(B)]
    for core in range(N_CORES):
        b_ = core // GROUPS
        g = core % GROUPS
        sl = slice(g * CS, (g + 1) * CS)
        wq = W_attn[sl, :]
        wk = W_attn[C + g * CS:C + (g + 1) * CS, :]
        wv = W_attn[2 * C + g * CS:2 * C + (g + 1) * CS, :]
        bqs = b_attn[sl]
        in_maps.append({
            "xT": xTb[b_].astype(ml_dtypes.bfloat16),
            "wqkT": np.ascontiguousarray(
                np.concatenate([wq, wk], 0).T).astype(ml_dtypes.bfloat16),
            "bq": np.ascontiguousarray(bqs.reshape(2, 128).T),
            "wvT": np.ascontiguousarray(wv.T).astype(ml_dtypes.bfloat16),
            "wpT": np.ascontiguousarray(
                W_proj[:, g * CS:(g + 1) * CS].T).astype(ml_dtypes.bfloat16),
        })
    return in_maps


_NC = None


def _get_nc():
    global _NC
    if _NC is None:
        _NC = build_nc()
    return _NC


def run(x, W_attn, b_attn, W_proj, b_proj, trace=False):
    nc = _get_nc()
    in_maps = make_in_maps(x, W_attn, b_attn, W_proj)
    res = run_bass_kernel_spmd(nc, in_maps, core_ids=list(range(N_CORES)),
                               trace=trace)
    out = np.zeros((B, T, C), dtype=np.float32)
    for core in range(N_CORES):
        out[core // GROUPS] += res.results[core]["outT"].T.astype(np.float32)
    # b_proj plus the folded-in v bias: y = P v + bv, sum(P)=1 per row
    b_eff = (np.asarray(b_proj, dtype=np.float32)
             + np.asarray(W_proj, dtype=np.float32)
             @ np.asarray(b_attn, dtype=np.float32)[2 * C:3 * C])
    out += b_eff[None, None, :]
    return out, res


def kernel(x, W_attn, b_attn, W_proj, b_proj):
    out, _ = run(x, W_attn, b_attn, W_proj, b_proj, trace=False)
    return out


# revision 22
# speedup vs baseline: 1.0346x; 1.0346x over previous
"""Causal self-attention (nn_CausalSelfAttention) on 8 TRN2 NeuronCores.

Reference computation (B=2, T=2048, C=1024, H=16 heads, D=64):
    qkv = x @ W_attn.T + b_attn ; split q,k,v
    y   = softmax(causal(q k^T / sqrt(D))) v        (per head)
    out = y @ W_proj.T + b_proj

Sharding: batch (2-way) x head-group (4-way, 4 heads each) -> 8 cores.
Each core computes its batch's attention for its 4 heads plus the partial
c_proj contribution of those heads' channels; the host sums the 4 partials
per batch (fp32) and adds the (adjusted) bias once.

Device-side simplifications (exact up to fp error):
  - k bias dropped (cancels in softmax); v bias folded into the host-side
    output bias (sum(P)=1 per row).

v3 design (PE tile-position concurrency + geometry-uniform batching):
  - S matmuls are K=64: the two sub-heads of a pair run as row tiles
    (0,0)/(64,0) adjacently -> ~2x PE throughput; per head-pair (pp) the
    S pair writes a [128, 2*512] two-bank PSUM tile and ONE fused ACT exp
    covers both heads (halves the ~293ns/instr ACT overhead). The two
    pps are staggered so S of one pp hides under the other pp's exp.
  - PV is M=64 col tiles (0,0)/(0,64) accumulating head pairs into one
    PSUM bank; denominators are 4x M=32 col-tiled chains (ones
    stationary) in one bank. PV+den are emitted in CHUNKS of 4 rounds:
    a chunk is a geometry-uniform burst (all deps long done -> single
    readiness wave), so the expensive PE geometry transitions (exposed
    drain + LDWEIGHTS, ~300ns each) are paid per chunk, not per round.
  - Production/projection chains drip between rounds; strip-0 production
    rotates over the (then idle) PV+den banks to double-buffer drains.
  - Final projection rotates over the S pair banks (idle by then), with
    drains split across Vector+Scalar; output is bf16 to halve the
    output DMA.

PSUM (8 banks): S pair tiles 2x2 + PV pairs 2 + denom 1 + production 1.
Known pitfalls: an accumulation chain covers all bytes it touches on its
first (start=True) matmul; custom DVE/gpsimd ops read partition 0 of
their input AP regardless of its base.
"""
import math
from contextlib import ExitStack

import ml_dtypes
import numpy as np

import concourse.bacc as bacc
import concourse.bass as bass
import concourse.mybir as mybir
import concourse.tile as tile
from concourse.bass_utils import run_bass_kernel_spmd

F32 = mybir.dt.float32
BF16 = mybir.dt.bfloat16
MMDT = BF16                    # dtype for all TensorE-facing tensors

N_CORES = 8
B, T, C, H = 2, 2048, 1024, 16
D = 64
GROUPS = N_CORES // B          # head groups per batch = 4
HPC = H // GROUPS              # heads per core = 4
CS = HPC * D                   # channel slice per core = 256
KT = C // 128                  # contraction tiles over C = 8
NS = T // 512                  # 512-wide query strips = 4
TT = T // 128                  # 128-row key tiles = 16
CH = 4                         # PV/den chunk size in rounds


def build_nc():
    nc = bacc.Bacc("TRN2", target_bir_lowering=False, debug=False,
                   num_devices=N_CORES)

    xT = nc.dram_tensor("xT", [C, T], MMDT, kind="ExternalInput")
    wqkT = nc.dram_tensor("wqkT", [C, 2 * CS], MMDT, kind="ExternalInput")
    bq = nc.dram_tensor("bq", [128, 2], F32, kind="ExternalInput")
    wvT = nc.dram_tensor("wvT", [C, CS], MMDT, kind="ExternalInput")
    wpT = nc.dram_tensor("wpT", [CS, C], MMDT, kind="ExternalInput")
    outT = nc.dram_tensor("outT", [C, T], BF16, kind="ExternalOutput")

    xTr = xT.ap().rearrange("(kt p) t -> kt p t", p=128)
    wqkr = wqkT.ap().rearrange("(kt p) n -> kt p n", p=128)
    wvr = wvT.ap().rearrange("(kt p) n -> kt p n", p=128)
    wpr = wpT.ap().rearrange("(kt p) n -> kt p n", p=128)

    scale = 1.0 / math.sqrt(D)

    with tile.TileContext(nc) as tc, ExitStack() as ctx:
        pw = ctx.enter_context(tc.tile_pool(name="pw", bufs=1))
        px = ctx.enter_context(tc.tile_pool(name="px", bufs=1))
        pq = ctx.enter_context(tc.tile_pool(name="pq", bufs=1))
        pk = ctx.enter_context(tc.tile_pool(name="pk", bufs=1))
        pv = ctx.enter_context(tc.tile_pool(name="pv", bufs=1))
        py = ctx.enter_context(tc.tile_pool(name="py", bufs=1))
        ppt = ctx.enter_context(tc.tile_pool(name="ppt", bufs=14))
        pnorm = ctx.enter_context(tc.tile_pool(name="pnorm", bufs=4))
        pout = ctx.enter_context(tc.tile_pool(name="pout", bufs=4))
        psS = ctx.enter_context(tc.tile_pool(name="psS", bufs=1, space="PSUM"))
        psPV = ctx.enter_context(tc.tile_pool(name="psPV", bufs=1,
                                              space="PSUM"))
        psD = ctx.enter_context(tc.tile_pool(name="psD", bufs=1, space="PSUM"))
        psP = ctx.enter_context(tc.tile_pool(name="psP", bufs=1, space="PSUM"))

        # ---- input DMA: per-tile 2D transfers, x strip0 + wqk first ----
        qs = [nc.sync, nc.scalar, nc.gpsimd]
        xq = [[None] * KT for _ in range(2)]   # [s][k] quarters, s in 0,1
        xh = [None] * KT                       # [k] cols 1024:2048
        wqk_sb, wv_sb = [], []
        qi = 0
        for k in range(KT):
            t_ = px.tile([128, 512], MMDT, tag=f"xq{k}_0", name=f"xq{k}_0")
            qs[qi % 3].dma_start(t_[:], xTr[k][:, 0:512])
            qi += 1
            xq[0][k] = t_
            wt = pw.tile([128, 2 * CS], MMDT, tag=f"wqk{k}", name=f"wqk{k}")
            qs[qi % 3].dma_start(wt[:], wqkr[k])
            qi += 1
            wqk_sb.append(wt)
        bq_sb = pw.tile([128, 2], F32, tag="bq", name="bq_sb")
        nc.gpsimd.dma_start(bq_sb[:], bq.ap())
        warm = pnorm.tile([128, 1], F32, tag="warm", name="warm")
        nc.scalar.activation(warm[:], bq_sb[:, 0:1],
                             mybir.ActivationFunctionType.Exp, scale=0.0)
        for k in range(KT):
            vt = pw.tile([128, CS], MMDT, tag=f"wv{k}", name=f"wv{k}")
            qs[(k + 1) % 3].dma_start(vt[:], wvr[k])
            wv_sb.append(vt)
        for k in range(KT):
            t_ = px.tile([128, 512], MMDT, tag=f"xq{k}_1", name=f"xq{k}_1")
            qs[k % 3].dma_start(t_[:], xTr[k][:, 512:1024])
            xq[1][k] = t_
        wp_sb = []
        for k2 in range(2):
            pt_ = pw.tile([128, C], MMDT, tag=f"wp{k2}", name=f"wp{k2}")
            nc.gpsimd.dma_start(pt_[:], wpr[k2])
            wp_sb.append(pt_)
        for k in range(KT):
            t_ = px.tile([128, 1024], MMDT, tag=f"xh{k}", name=f"xh{k}")
            qs[k % 2].dma_start(t_[:], xTr[k][:, 1024:2048])
            xh[k] = t_

        ones_sb = pw.tile([128, 32], MMDT, tag="ones", name="ones_sb")
        nc.vector.memset(ones_sb[:], 1.0)

        def x_strip(k, s):
            if s < 2:
                return xq[s][k][:]
            return xh[k][:, (s - 2) * 512:(s - 1) * 512]

        # persistent SBUF tensors (pair layout: sub-head r at partitions
        # r*64..(r+1)*64)
        q_sb = [pq.tile([128, T], MMDT, tag=f"q{pp}", name=f"q{pp}")
                for pp in range(2)]
        k_sb = [pk.tile([128, T], MMDT, tag=f"k{pp}", name=f"k{pp}")
                for pp in range(2)]
        v_sb = [pv.tile([128, CS], MMDT, tag=f"v{n}", name=f"v{n}")
                for n in range(TT)]
        y_sb = [py.tile([128, T], MMDT, tag=f"y{pp}", name=f"y{pp}")
                for pp in range(2)]

        def unit_k(s, pp, pool=None, tag="pm"):
            ps = (pool or psP).tile([128, 512], F32, tag=tag, name="ps_k")
            steps = []
            for k in range(KT):
                steps.append(lambda k=k, ps=ps: nc.tensor.matmul(
                    ps[:],
                    wqk_sb[k][:, (2 + pp) * 128:(3 + pp) * 128],
                    x_strip(k, s),
                    start=(k == 0), stop=(k == KT - 1),
                ))
            steps.append(lambda ps=ps: nc.vector.tensor_copy(
                k_sb[pp][:, s * 512:(s + 1) * 512], ps[:]))
            return steps

        def unit_q(s, pp, pool=None, tag="pm"):
            ps = (pool or psP).tile([128, 512], F32, tag=tag, name="ps_q")
            steps = []
            for k in range(KT):
                steps.append(lambda k=k, ps=ps: nc.tensor.matmul(
                    ps[:],
                    wqk_sb[k][:, pp * 128:(pp + 1) * 128],
                    x_strip(k, s),
                    start=(k == 0), stop=(k == KT - 1),
                ))
            steps.append(lambda ps=ps: nc.vector.tensor_scalar(
                q_sb[pp][:, s * 512:(s + 1) * 512], ps[:],
                bq_sb[:, pp:pp + 1], None, op0=mybir.AluOpType.add))
            return steps

        def unit_v(s, j, pool=None, tag="pm"):
            n = 4 * s + j
            ps = (pool or psP).tile([128, 512], F32, tag=tag, name="ps_v")
            steps = []
            for k in range(KT):
                steps.append(lambda k=k, ps=ps: nc.tensor.matmul(
                    ps[:, 0:CS],
                    x_strip(k, s)[:, j * 128:(j + 1) * 128],
                    wv_sb[k][:],
                    start=(k == 0), stop=(k == KT - 1),
                ))
            steps.append(lambda ps=ps, n=n: nc.vector.tensor_copy(
                v_sb[n][:], ps[:, 0:CS]))
            return steps

        def unit_proj(s, ct, pool=None, tag="pm", split_drain=False):
            ps = (pool or psP).tile([128, 512], F32, tag=tag, name="ps_o")
            steps = []
            for k2 in range(2):
                steps.append(lambda k2=k2, ps=ps: nc.tensor.matmul(
                    ps[:],
                    wp_sb[k2][:, ct * 128:(ct + 1) * 128],
                    y_sb[k2][:, s * 512:(s + 1) * 512],
                    start=(k2 == 0), stop=(k2 == 1),
                ))

            def drain(ps=ps, s=s, ct=ct):
                ot = pout.tile([128, 512], BF16, tag="ot", name="ot")
                if split_drain:
                    nc.vector.tensor_copy(ot[:, 0:256], ps[:, 0:256])
                    nc.scalar.copy(ot[:, 256:512], ps[:, 256:512])
                    eng = [nc.sync, nc.scalar][ct % 2]
                else:
                    nc.vector.tensor_copy(ot[:], ps[:])
                    eng = nc.sync if ct % 2 == 0 else nc.gpsimd
                eng.dma_start(
                    outT.ap()[ct * 128:(ct + 1) * 128,
                              s * 512:(s + 1) * 512],
                    ot[:])
            steps.append(drain)
            return steps

        def kq_units(s, pools=None):
            us, pl, i = [], pools or [(None, "pm")], 0
            for pp in range(2):
                po_, tg = pl[i % len(pl)]; i += 1
                us.append(unit_k(s, pp, pool=po_, tag=tg))
            for pp in range(2):
                po_, tg = pl[i % len(pl)]; i += 1
                us.append(unit_q(s, pp, pool=po_, tag=tg))
            return us

        def v_units(s, pools=None):
            us, pl, i = [], pools or [(None, "pm")], 0
            for j in range(4):
                po_, tg = pl[i % len(pl)]; i += 1
                us.append(unit_v(s, j, pool=po_, tag=tg))
            return us

        def run_units(units, count):
            done = 0
            while units and done < count:
                for f in units.pop(0):
                    f()
                done += 1

        # strip-0 production rotates over psP + (idle) psPV/psD banks
        s0pools = [(psP, "pm"), (psPV, "pv0"), (psPV, "pv1"), (psD, "den")]
        for u in kq_units(0, pools=s0pools) + v_units(0, pools=s0pools):
            for f in u:
                f()

        for s in range(NS):
            # drip load-balanced by strip capacity: v(s) lands in this
            # strip's first rounds (first PV chunk needs it at round 4),
            # kq(s+1) by strip end, projection deferred to late strips.
            bg = []
            if s >= 1:
                bg += v_units(s)
            if s + 1 < NS:
                pools = s0pools if s == 0 else None
                bg += kq_units(s + 1, pools=pools)
            if s == 2:
                bg += [unit_proj(0, ct) for ct in range(8)]
            if s == 3:
                bg += [unit_proj(1, ct) for ct in range(8)]
                bg += [unit_proj(2, ct) for ct in range(8)]
            nbg = len(bg)
            nt = 4 * s + 4
            pv_ps = [psPV.tile([128, 512], F32, tag=f"pv{pp}",
                               name=f"pv{pp}") for pp in range(2)]
            den_ps = psD.tile([128, 512], F32, tag="den", name="den_ps")
            emitted = 0
            pend = []          # (n, off, pts) awaiting chunk emission

            def emit_chunk(keep=0):
                # geometry-uniform burst: all PV col-tile pairs, then all
                # den groups; every dep (exp) completed rounds ago. `keep`
                # holds back the freshest rounds whose exps may still be
                # in flight.
                batch = pend[:len(pend) - keep] if keep else pend[:]
                del pend[:len(batch)]
                for (n, off, pts) in batch:
                    for pp in range(2):
                        for r in range(2):
                            nc.tensor.matmul(
                                pv_ps[pp][r * 64:(r + 1) * 64, off:512],
                                v_sb[n][:, (2 * pp + r) * 64:
                                        (2 * pp + r + 1) * 64],
                                pts[pp][:, r, off:512],
                                start=(n == 0), stop=(n == nt - 1),
                            )
                for (n, off, pts) in batch:
                    for h4 in range(4):
                        nc.tensor.matmul(
                            den_ps[32 * h4:32 * (h4 + 1), off:512],
                            ones_sb[:],
                            pts[h4 // 2][:, h4 % 2, off:512],
                            start=(n == 0), stop=(n == nt - 1),
                            tile_position=(0, 32 * h4),
                        )

            for n in range(nt):
                if n % CH == 0 and n > 0:
                    emit_chunk(keep=2)
                target = (n * nbg) // nt if s == 0 else ((n + 1) * nbg) // nt
                run_units(bg, target - emitted)
                emitted = min(target, nbg)
                off = max(0, n - 4 * s) * 128
                pts = []
                for pp in range(2):
                    st = psS.tile([128, 2 * 512], F32, tag=f"s{pp}",
                                  name=f"st{pp}")
                    st3 = st.rearrange("p (r m) -> p r m", m=512)
                    for r in range(2):
                        nc.tensor.matmul(
                            st3[:, r, off:512],
                            k_sb[pp][r * 64:(r + 1) * 64,
                                     n * 128:(n + 1) * 128],
                            q_sb[pp][r * 64:(r + 1) * 64,
                                     s * 512 + off:(s + 1) * 512],
                            start=True, stop=True,
                        )
                    pt = ppt.tile([128, 2 * 512], MMDT, tag="pt", name="pt")
                    pt3 = pt.rearrange("p (r m) -> p r m", m=512)
                    nc.scalar.activation(
                        pt3[:, :, off:512], st3[:, :, off:512],
                        mybir.ActivationFunctionType.Exp, scale=scale)
                    if n >= 4 * s:
                        for r in range(2):
                            nc.gpsimd.affine_select(
                                out=pt3[:, r, off:off + 128],
                                in_=pt3[:, r, off:off + 128],
                                compare_op=mybir.AluOpType.is_ge,
                                fill=0.0, base=0,
                                pattern=[[1, 128]], channel_multiplier=-1)
                    pts.append(pt3)
                pend.append((n, off, pts))

            emit_chunk()
            run_units(bg, 10**9)

            # ---- normalize: y = y_unnorm / denom ----
            rrs, rbs = [], []
            for h4 in range(4):
                dtmp = pnorm.tile([1, 512], F32, tag="dtmp", name="dtmp")
                nc.vector.tensor_copy(dtmp[:], den_ps[32 * h4:32 * h4 + 1, :])
                rr = pnorm.tile([1, 512], F32, tag="rr", name="rr")
                nc.vector.reciprocal_approx_fast(rr[:], dtmp[:])
                rrs.append(rr)
            for pp in range(2):
                for r in range(2):
                    h4 = 2 * pp + r
                    rb = pnorm.tile([64, 512], F32, tag="rb", name="rb")
                    nc.gpsimd.partition_broadcast(rb[:], rrs[h4][:])
                    rbs.append(rb)
                for r in range(2):
                    h4 = 2 * pp + r
                    nc.vector.tensor_tensor(
                        y_sb[pp][r * 64:(r + 1) * 64, s * 512:(s + 1) * 512],
                        pv_ps[pp][r * 64:(r + 1) * 64, :], rbs[h4][:],
                        op=mybir.AluOpType.mult)

        # final projection: rotate over S pair banks (idle now) + psP
        tailpools = [(psS, "s0"), (psS, "s1"), (psP, "pm")]
        for ct in range(8):
            po_, tg = tailpools[ct % 3]
            for f in unit_proj(NS - 1, ct, pool=po_, tag=tg,
                               split_drain=True):
                f()

    nc.compile()
    return nc


def make_in_maps(x, W_attn, b_attn, W_proj):
    """Shard full inputs into the 8 per-core input dicts."""
    x = np.asarray(x, dtype=np.float32)
    W_attn = np.asarray(W_attn, dtype=np.float32)
    b_attn = np.asarray(b_attn, dtype=np.float32)
    W_proj = np.asarray(W_proj, dtype=np.float32)
    in_maps = []
    xTb = [np.ascontiguousarray(x[b_].T) for b_ in range(B)]
    for core in range(N_CORES):
        b_ = core // GROUPS
        g = core % GROUPS
        sl = slice(g * CS, (g + 1) * CS)
        wq = W_attn[sl, :]
        wk = W_attn[C + g * CS:C + (g + 1) * CS, :]
        wv = W_attn[2 * C + g * CS:2 * C + (g + 1) * CS, :]
        bqs = b_attn[sl]
        in_maps.append({
            "xT": xTb[b_].astype(ml_dtypes.bfloat16),
            "wqkT": np.ascontiguousarray(
                np.concatenate([wq, wk], 0).T).astype(ml_dtypes.bfloat16),
            "bq": np.ascontiguousarray(bqs.reshape(2, 128).T),
            "wvT": np.ascontiguousarray(wv.T).astype(ml_dtypes.bfloat16),
            "wpT": np.ascontiguousarray(
                W_proj[:, g * CS:(g + 1) * CS].T).astype(ml_dtypes.bfloat16),
        })
    return in_maps


_NC = None


def _get_nc():
    global _NC
    if _NC is None:
        _NC = build_nc()
    return _NC


def run(x, W_attn, b_attn, W_proj, b_proj, trace=False):
    nc = _get_nc()
    in_maps = make_in_maps(x, W_attn, b_attn, W_proj)
    res = run_bass_kernel_spmd(nc, in_maps, core_ids=list(range(N_CORES)),
                               trace=trace)
    out = np.zeros((B, T, C), dtype=np.float32)
    for core in range(N_CORES):
        out[core // GROUPS] += res.results[core]["outT"].T.astype(np.float32)
    # b_proj plus the folded-in v bias: y = P v + bv, sum(P)=1 per row
    b_eff = (np.asarray(b_proj, dtype=np.float32)
             + np.asarray(W_proj, dtype=np.float32)
             @ np.asarray(b_attn, dtype=np.float32)[2 * C:3 * C])
    out += b_eff[None, None, :]
    return out, res


def kernel(x, W_attn, b_attn, W_proj, b_proj):
    out, _ = run(x, W_attn, b_attn, W_proj, b_proj, trace=False)
    return out


# revision 23
# speedup vs baseline: 1.0349x; 1.0004x over previous
"""Causal self-attention (nn_CausalSelfAttention) on 8 TRN2 NeuronCores.

Reference computation (B=2, T=2048, C=1024, H=16 heads, D=64):
    qkv = x @ W_attn.T + b_attn ; split q,k,v
    y   = softmax(causal(q k^T / sqrt(D))) v        (per head)
    out = y @ W_proj.T + b_proj

Sharding: batch (2-way) x head-group (4-way, 4 heads each) -> 8 cores.
Each core computes its batch's attention for its 4 heads plus the partial
c_proj contribution of those heads' channels; the host sums the 4 partials
per batch (fp32) and adds the (adjusted) bias once.

Device-side simplifications (exact up to fp error):
  - k bias dropped (cancels in softmax); v bias folded into the host-side
    output bias (sum(P)=1 per row).

v3 design (PE tile-position concurrency + geometry-uniform batching):
  - S matmuls are K=64: the two sub-heads of a pair run as row tiles
    (0,0)/(64,0) adjacently -> ~2x PE throughput; per head-pair (pp) the
    S pair writes a [128, 2*512] two-bank PSUM tile and ONE fused ACT exp
    covers both heads (halves the ~293ns/instr ACT overhead). The two
    pps are staggered so S of one pp hides under the other pp's exp.
  - PV is M=64 col tiles (0,0)/(0,64) accumulating head pairs into one
    PSUM bank; denominators are 4x M=32 col-tiled chains (ones
    stationary) in one bank. PV+den are emitted in CHUNKS of 4 rounds:
    a chunk is a geometry-uniform burst (all deps long done -> single
    readiness wave), so the expensive PE geometry transitions (exposed
    drain + LDWEIGHTS, ~300ns each) are paid per chunk, not per round.
  - Production/projection chains drip between rounds; strip-0 production
    rotates over the (then idle) PV+den banks to double-buffer drains.
  - Final projection rotates over the S pair banks (idle by then), with
    drains split across Vector+Scalar; output is bf16 to halve the
    output DMA.

PSUM (8 banks): S pair tiles 2x2 + PV pairs 2 + denom 1 + production 1.
Known pitfalls: an accumulation chain covers all bytes it touches on its
first (start=True) matmul; custom DVE/gpsimd ops read partition 0 of
their input AP regardless of its base.
"""
import math
from contextlib import ExitStack

import ml_dtypes
import numpy as np

import concourse.bacc as bacc
import concourse.bass as bass
import concourse.mybir as mybir
import concourse.tile as tile
from concourse.bass_utils import run_bass_kernel_spmd

F32 = mybir.dt.float32
BF16 = mybir.dt.bfloat16
MMDT = BF16                    # dtype for all TensorE-facing tensors

N_CORES = 8
B, T, C, H = 2, 2048, 1024, 16
D = 64
GROUPS = N_CORES // B          # head groups per batch = 4
HPC = H // GROUPS              # heads per core = 4
CS = HPC * D                   # channel slice per core = 256
KT = C // 128                  # contraction tiles over C = 8
NS = T // 512                  # 512-wide query strips = 4
TT = T // 128                  # 128-row key tiles = 16
CH = 4                         # PV/den chunk size in rounds


def build_nc():
    nc = bacc.Bacc("TRN2", target_bir_lowering=False, debug=False,
                   num_devices=N_CORES)

    xT = nc.dram_tensor("xT", [C, T], MMDT, kind="ExternalInput")
    wqkT = nc.dram_tensor("wqkT", [C, 2 * CS], MMDT, kind="ExternalInput")
    bq = nc.dram_tensor("bq", [128, 2], F32, kind="ExternalInput")
    wvT = nc.dram_tensor("wvT", [C, CS], MMDT, kind="ExternalInput")
    wpT = nc.dram_tensor("wpT", [CS, C], MMDT, kind="ExternalInput")
    outT = nc.dram_tensor("outT", [C, T], BF16, kind="ExternalOutput")

    xTr = xT.ap().rearrange("(kt p) t -> kt p t", p=128)
    wqkr = wqkT.ap().rearrange("(kt p) n -> kt p n", p=128)
    wvr = wvT.ap().rearrange("(kt p) n -> kt p n", p=128)
    wpr = wpT.ap().rearrange("(kt p) n -> kt p n", p=128)

    scale = 1.0 / math.sqrt(D)

    with tile.TileContext(nc) as tc, ExitStack() as ctx:
        pw = ctx.enter_context(tc.tile_pool(name="pw", bufs=1))
        px = ctx.enter_context(tc.tile_pool(name="px", bufs=1))
        pq = ctx.enter_context(tc.tile_pool(name="pq", bufs=1))
        pk = ctx.enter_context(tc.tile_pool(name="pk", bufs=1))
        pv = ctx.enter_context(tc.tile_pool(name="pv", bufs=1))
        py = ctx.enter_context(tc.tile_pool(name="py", bufs=1))
        ppt = ctx.enter_context(tc.tile_pool(name="ppt", bufs=14))
        pnorm = ctx.enter_context(tc.tile_pool(name="pnorm", bufs=4))
        pout = ctx.enter_context(tc.tile_pool(name="pout", bufs=4))
        psS = ctx.enter_context(tc.tile_pool(name="psS", bufs=1, space="PSUM"))
        psPV = ctx.enter_context(tc.tile_pool(name="psPV", bufs=1,
                                              space="PSUM"))
        psD = ctx.enter_context(tc.tile_pool(name="psD", bufs=1, space="PSUM"))
        psP = ctx.enter_context(tc.tile_pool(name="psP", bufs=1, space="PSUM"))

        # ---- input DMA: per-tile 2D transfers, x strip0 + wqk first ----
        qs = [nc.sync, nc.scalar, nc.gpsimd]
        xq = [[None] * KT for _ in range(2)]   # [s][k] quarters, s in 0,1
        xh = [None] * KT                       # [k] cols 1024:2048
        wqk_sb, wv_sb = [], []
        qi = 0
        for k in range(KT):
            t_ = px.tile([128, 512], MMDT, tag=f"xq{k}_0", name=f"xq{k}_0")
            qs[qi % 3].dma_start(t_[:], xTr[k][:, 0:512])
            qi += 1
            xq[0][k] = t_
            wt = pw.tile([128, 2 * CS], MMDT, tag=f"wqk{k}", name=f"wqk{k}")
            qs[qi % 3].dma_start(wt[:], wqkr[k])
            qi += 1
            wqk_sb.append(wt)
        bq_sb = pw.tile([128, 2], F32, tag="bq", name="bq_sb")
        nc.gpsimd.dma_start(bq_sb[:], bq.ap())
        warm = pnorm.tile([128, 1], F32, tag="warm", name="warm")
        nc.scalar.activation(warm[:], bq_sb[:, 0:1],
                             mybir.ActivationFunctionType.Exp, scale=0.0)
        for k in range(KT):
            vt = pw.tile([128, CS], MMDT, tag=f"wv{k}", name=f"wv{k}")
            qs[(k + 1) % 3].dma_start(vt[:], wvr[k])
            wv_sb.append(vt)
        for k in range(KT):
            t_ = px.tile([128, 512], MMDT, tag=f"xq{k}_1", name=f"xq{k}_1")
            qs[k % 3].dma_start(t_[:], xTr[k][:, 512:1024])
            xq[1][k] = t_
        wp_sb = []
        for k2 in range(2):
            pt_ = pw.tile([128, C], MMDT, tag=f"wp{k2}", name=f"wp{k2}")
            nc.gpsimd.dma_start(pt_[:], wpr[k2])
            wp_sb.append(pt_)
        for k in range(KT):
            t_ = px.tile([128, 1024], MMDT, tag=f"xh{k}", name=f"xh{k}")
            qs[k % 2].dma_start(t_[:], xTr[k][:, 1024:2048])
            xh[k] = t_

        ones_sb = pw.tile([128, 32], MMDT, tag="ones", name="ones_sb")
        nc.vector.memset(ones_sb[:], 1.0)

        def x_strip(k, s):
            if s < 2:
                return xq[s][k][:]
            return xh[k][:, (s - 2) * 512:(s - 1) * 512]

        # persistent SBUF tensors (pair layout: sub-head r at partitions
        # r*64..(r+1)*64)
        q_sb = [pq.tile([128, T], MMDT, tag=f"q{pp}", name=f"q{pp}")
                for pp in range(2)]
        k_sb = [pk.tile([128, T], MMDT, tag=f"k{pp}", name=f"k{pp}")
                for pp in range(2)]
        v_sb = [pv.tile([128, CS], MMDT, tag=f"v{n}", name=f"v{n}")
                for n in range(TT)]
        y_sb = [py.tile([128, T], MMDT, tag=f"y{pp}", name=f"y{pp}")
                for pp in range(2)]

        def unit_k(s, pp, pool=None, tag="pm"):
            ps = (pool or psP).tile([128, 512], F32, tag=tag, name="ps_k")
            steps = []
            for k in range(KT):
                steps.append(lambda k=k, ps=ps: nc.tensor.matmul(
                    ps[:],
                    wqk_sb[k][:, (2 + pp) * 128:(3 + pp) * 128],
                    x_strip(k, s),
                    start=(k == 0), stop=(k == KT - 1),
                ))
            steps.append(lambda ps=ps: nc.vector.tensor_copy(
                k_sb[pp][:, s * 512:(s + 1) * 512], ps[:]))
            return steps

        def unit_q(s, pp, pool=None, tag="pm"):
            ps = (pool or psP).tile([128, 512], F32, tag=tag, name="ps_q")
            steps = []
            for k in range(KT):
                steps.append(lambda k=k, ps=ps: nc.tensor.matmul(
                    ps[:],
                    wqk_sb[k][:, pp * 128:(pp + 1) * 128],
                    x_strip(k, s),
                    start=(k == 0), stop=(k == KT - 1),
                ))
            steps.append(lambda ps=ps: nc.vector.tensor_scalar(
                q_sb[pp][:, s * 512:(s + 1) * 512], ps[:],
                bq_sb[:, pp:pp + 1], None, op0=mybir.AluOpType.add))
            return steps

        def unit_v(s, j, pool=None, tag="pm"):
            n = 4 * s + j
            ps = (pool or psP).tile([128, 512], F32, tag=tag, name="ps_v")
            steps = []
            for k in range(KT):
                steps.append(lambda k=k, ps=ps: nc.tensor.matmul(
                    ps[:, 0:CS],
                    x_strip(k, s)[:, j * 128:(j + 1) * 128],
                    wv_sb[k][:],
                    start=(k == 0), stop=(k == KT - 1),
                ))
            steps.append(lambda ps=ps, n=n: nc.vector.tensor_copy(
                v_sb[n][:], ps[:, 0:CS]))
            return steps

        def unit_proj(s, ct, pool=None, tag="pm", split_drain=False):
            ps = (pool or psP).tile([128, 512], F32, tag=tag, name="ps_o")
            steps = []
            for k2 in range(2):
                steps.append(lambda k2=k2, ps=ps: nc.tensor.matmul(
                    ps[:],
                    wp_sb[k2][:, ct * 128:(ct + 1) * 128],
                    y_sb[k2][:, s * 512:(s + 1) * 512],
                    start=(k2 == 0), stop=(k2 == 1),
                ))

            def drain(ps=ps, s=s, ct=ct):
                ot = pout.tile([128, 512], BF16, tag="ot", name="ot")
                if split_drain:
                    nc.vector.tensor_copy(ot[:, 0:256], ps[:, 0:256])
                    nc.scalar.copy(ot[:, 256:512], ps[:, 256:512])
                    eng = [nc.sync, nc.scalar][ct % 2]
                else:
                    nc.vector.tensor_copy(ot[:], ps[:])
                    eng = nc.sync if ct % 2 == 0 else nc.gpsimd
                eng.dma_start(
                    outT.ap()[ct * 128:(ct + 1) * 128,
                              s * 512:(s + 1) * 512],
                    ot[:])
            steps.append(drain)
            return steps

        def kq_units(s, pools=None):
            us, pl, i = [], pools or [(None, "pm")], 0
            for pp in range(2):
                po_, tg = pl[i % len(pl)]; i += 1
                us.append(unit_k(s, pp, pool=po_, tag=tg))
            for pp in range(2):
                po_, tg = pl[i % len(pl)]; i += 1
                us.append(unit_q(s, pp, pool=po_, tag=tg))
            return us

        def v_units(s, pools=None):
            us, pl, i = [], pools or [(None, "pm")], 0
            for j in range(4):
                po_, tg = pl[i % len(pl)]; i += 1
                us.append(unit_v(s, j, pool=po_, tag=tg))
            return us

        def run_units(units, count):
            done = 0
            while units and done < count:
                for f in units.pop(0):
                    f()
                done += 1

        # strip-0 production rotates over psP + (idle) psPV/psD banks
        s0pools = [(psP, "pm"), (psPV, "pv0"), (psPV, "pv1"), (psD, "den")]
        for u in kq_units(0, pools=s0pools) + v_units(0, pools=s0pools):
            for f in u:
                f()

        for s in range(NS):
            # drip load-balanced by strip capacity: v(s) lands in this
            # strip's first rounds (first PV chunk needs it at round 4),
            # kq(s+1) by strip end, projection deferred to late strips.
            bg = []
            if s >= 1:
                bg += v_units(s)
            if s + 1 < NS:
                pools = s0pools if s == 0 else None
                bg += kq_units(s + 1, pools=pools)
            if s == 2:
                bg += [unit_proj(0, ct) for ct in range(8)]
            if s == 3:
                bg += [unit_proj(1, ct) for ct in range(8)]
                bg += [unit_proj(2, ct) for ct in range(8)]
            nbg = len(bg)
            nt = 4 * s + 4
            pv_ps = [psPV.tile([128, 512], F32, tag=f"pv{pp}",
                               name=f"pv{pp}") for pp in range(2)]
            den_ps = psD.tile([128, 512], F32, tag="den", name="den_ps")
            emitted = 0
            pend = []          # (n, off, pts) awaiting chunk emission

            def emit_chunk(keep=0):
                # geometry-uniform burst: all PV col-tile pairs, then all
                # den groups; every dep (exp) completed rounds ago. `keep`
                # holds back the freshest rounds whose exps may still be
                # in flight.
                batch = pend[:len(pend) - keep] if keep else pend[:]
                del pend[:len(batch)]
                for (n, off, pts) in batch:
                    for pp in range(2):
                        for r in range(2):
                            nc.tensor.matmul(
                                pv_ps[pp][r * 64:(r + 1) * 64, off:512],
                                v_sb[n][:, (2 * pp + r) * 64:
                                        (2 * pp + r + 1) * 64],
                                pts[pp][:, r, off:512],
                                start=(n == 0), stop=(n == nt - 1),
                            )
                for (n, off, pts) in batch:
                    for h4 in range(4):
                        nc.tensor.matmul(
                            den_ps[32 * h4:32 * (h4 + 1), off:512],
                            ones_sb[:],
                            pts[h4 // 2][:, h4 % 2, off:512],
                            start=(n == 0), stop=(n == nt - 1),
                            tile_position=(0, 32 * h4),
                        )

            for n in range(nt):
                if n % CH == 0 and n > 0:
                    emit_chunk(keep=2)
                target = (n * nbg) // nt if s == 0 else ((n + 1) * nbg) // nt
                run_units(bg, target - emitted)
                emitted = min(target, nbg)
                off = max(0, n - 4 * s) * 128
                pts = []
                for pp in range(2):
                    st = psS.tile([128, 2 * 512], F32, tag=f"s{pp}",
                                  name=f"st{pp}")
                    st3 = st.rearrange("p (r m) -> p r m", m=512)
                    for r in range(2):
                        nc.tensor.matmul(
                            st3[:, r, off:512],
                            k_sb[pp][r * 64:(r + 1) * 64,
                                     n * 128:(n + 1) * 128],
                            q_sb[pp][r * 64:(r + 1) * 64,
                                     s * 512 + off:(s + 1) * 512],
                            start=True, stop=True,
                        )
                    pt = ppt.tile([128, 2 * 512], MMDT, tag="pt", name="pt")
                    pt3 = pt.rearrange("p (r m) -> p r m", m=512)
                    if off == 0:
                        # contiguous 2D AP is measurably faster on ACT than
                        # the equivalent strided 3D form
                        nc.scalar.activation(
                            pt[:, 0:1024], st[:, 0:1024],
                            mybir.ActivationFunctionType.Exp, scale=scale)
                    else:
                        nc.scalar.activation(
                            pt3[:, :, off:512], st3[:, :, off:512],
                            mybir.ActivationFunctionType.Exp, scale=scale)
                    if n >= 4 * s:
                        for r in range(2):
                            nc.gpsimd.affine_select(
                                out=pt3[:, r, off:off + 128],
                                in_=pt3[:, r, off:off + 128],
                                compare_op=mybir.AluOpType.is_ge,
                                fill=0.0, base=0,
                                pattern=[[1, 128]], channel_multiplier=-1)
                    pts.append(pt3)
                pend.append((n, off, pts))

            emit_chunk()
            run_units(bg, 10**9)

            # ---- normalize: y = y_unnorm / denom ----
            rrs, rbs = [], []
            for h4 in range(4):
                dtmp = pnorm.tile([1, 512], F32, tag="dtmp", name="dtmp")
                nc.vector.tensor_copy(dtmp[:], den_ps[32 * h4:32 * h4 + 1, :])
                rr = pnorm.tile([1, 512], F32, tag="rr", name="rr")
                nc.vector.reciprocal_approx_fast(rr[:], dtmp[:])
                rrs.append(rr)
            for pp in range(2):
                for r in range(2):
                    h4 = 2 * pp + r
                    rb = pnorm.tile([64, 512], F32, tag="rb", name="rb")
                    nc.gpsimd.partition_broadcast(rb[:], rrs[h4][:])
                    rbs.append(rb)
                for r in range(2):
                    h4 = 2 * pp + r
                    nc.vector.tensor_tensor(
                        y_sb[pp][r * 64:(r + 1) * 64, s * 512:(s + 1) * 512],
                        pv_ps[pp][r * 64:(r + 1) * 64, :], rbs[h4][:],
                        op=mybir.AluOpType.mult)

        # final projection: rotate over S pair banks (idle now) + psP
        tailpools = [(psS, "s0"), (psS, "s1"), (psP, "pm")]
        for ct in range(8):
            po_, tg = tailpools[ct % 3]
            for f in unit_proj(NS - 1, ct, pool=po_, tag=tg,
                               split_drain=True):
                f()

    nc.compile()
    return nc


def make_in_maps(x, W_attn, b_attn, W_proj):
    """Shard full inputs into the 8 per-core input dicts."""
    x = np.asarray(x, dtype=np.float32)
    W_attn = np.asarray(W_attn, dtype=np.float32)
    b_attn = np.asarray(b_attn, dtype=np.float32)
    W_proj = np.asarray(W_proj, dtype=np.float32)
    in_maps = []
    xTb = [np.ascontiguousarray(x[b_].T) for b_ in range(B)]
    for core in range(N_CORES):
        b_ = core // GROUPS
        g = core % GROUPS
        sl = slice(g * CS, (g + 1) * CS)
        wq = W_attn[sl, :]
        wk = W_attn[C + g * CS:C + (g + 1) * CS, :]
        wv = W_attn[2 * C + g * CS:2 * C + (g + 1) * CS, :]
        bqs = b_attn[sl]
        in_maps.append({
            "xT": xTb[b_].astype(ml_dtypes.bfloat16),
            "wqkT": np.ascontiguousarray(
                np.concatenate([wq, wk], 0).T).astype(ml_dtypes.bfloat16),
            "bq": np.ascontiguousarray(bqs.reshape(2, 128).T),
            "wvT": np.ascontiguousarray(wv.T).astype(ml_dtypes.bfloat16),
            "wpT": np.ascontiguousarray(
                W_proj[:, g * CS:(g + 1) * CS].T).astype(ml_dtypes.bfloat16),
        })
    return in_maps


_NC = None


def _get_nc():
    global _NC
    if _NC is None:
        _NC = build_nc()
    return _NC


def run(x, W_attn, b_attn, W_proj, b_proj, trace=False):
    nc = _get_nc()
    in_maps = make_in_maps(x, W_attn, b_attn, W_proj)
    res = run_bass_kernel_spmd(nc, in_maps, core_ids=list(range(N_CORES)),
                               trace=trace)
    out = np.zeros((B, T, C), dtype=np.float32)
    for core in range(N_CORES):
        out[core // GROUPS] += res.results[core]["outT"].T.astype(np.float32)
    # b_proj plus the folded-in v bias: y = P v + bv, sum(P)=1 per row
    b_eff = (np.asarray(b_proj, dtype=np.float32)
             + np.asarray(W_proj, dtype=np.float32)
             @ np.asarray(b_attn, dtype=np.float32)[2 * C:3 * C])
    out += b_eff[None, None, :]
    return out, res


def kernel(x, W_attn, b_attn, W_proj, b_proj):
    out, _ = run(x, W_attn, b_attn, W_proj, b_proj, trace=False)
    return out
